# revision 27
# baseline (speedup 1.0000x reference)
"""GAT (graph attention network) Bass kernel for 8 trn2 NeuronCores.

Strategy (row-sharding): core k owns query rows [k*512, (k+1)*512).
 - Every core computes Wh = x @ W[h] for ALL nodes (replicated, cheap on PE)
   in [node-on-partition, feature] layout; s1 = x@(W a1) and s2 = x@(W a2)
   come from one thin matmul against a packed [wa1|wa2] weight block.
   Wh is batched: ALL 8 heads accumulate into one PSUM bank per node-block
   (k-major weight layout), one f32->bf16 cast per block.
 - Hidden attention per head, transposed layout [keys j on partitions,
   own rows i free]: us = leaky(nm + s1[i] + s2[j]) is ONE custom DVE op
   per j-block (4 ALU stages: add, add, mul-alpha, max), registered into
   the per-NEFF DVE table at import. This removes the separate z-base
   tensor_tensor, the per-block +s2 tensor_scalar, and the ScalarE Prelu
   entirely -- ScalarE does exp only. out_head.T = [Wh|1].T @ P
   accumulated in PSUM gives both att@Wh and softmax denominators.
 - Per-head normalize: 1/den via exp(-ln(den)) on ScalarE (reads PSUM
   directly), then mult + elu -> h kept transposed as lhsT for
   Wh_o = h @ W_out. The Wh_o partial-sum AllGather is split 4 ways:
   c-blocks {0,1} at head 4, {2} at head 6, head-6's half of c-block 3
   (K=64 matmul) during head 7, so only head-7's half (~35KB) plus a tiny
   s2o-column gather sit on the critical tail.
 - Output attention layer runs the same fused-op way (s0 = gathered s2o),
   followed by elu + log_softmax.
"""

import sys

sys.path.insert(0, "/opt/trn_rl_repo")

import numpy as np
import ml_dtypes

import concourse.bass as bass
import concourse.bacc as bacc
import concourse.tile as tile
from concourse import mybir
from concourse.bass_utils import run_bass_kernel_spmd
from concourse.masks import make_identity

F32 = mybir.dt.float32
BF16 = mybir.dt.bfloat16
BF = ml_dtypes.bfloat16
ADD = mybir.AluOpType.add
MULT = mybir.AluOpType.mult
MAX = mybir.AluOpType.max
AF = mybir.ActivationFunctionType

# problem shape (hardcoded per spec)
N = 4096
F_IN = 512
O = 64
H = 8
C = 16
N_CORES = 8
NEG = -100.0  # additive mask offset; exp(leaky(-100+e)) <= ~1e-8
ALPHA = 0.2

# knobs
GROUP = 8          # j-blocks per activation slab
S_BLOCKS = 1       # per-slab blocks routed {TT on V + bias-Prelu on S}
                   # instead of the fused custom op (S/V balance fine-tune)
P_BLOCKS = 0       # per-slab blocks with the z-base TT on GpSimd instead of V
KF = F_IN // 128   # f_in k-tiles


# ---- custom DVE ops: fused us = max(t, alpha*t), t = in0 (+ in1) + s0 ----
def _register_dve_ops():
    from concourse import dve_ops as dvo
    from concourse.dve_spec import Spec, Src0, Src1, C0, C2, maxx, lower
    from concourse.dve_uop import DveOpSpec

    def reg(name, spec, rd1):
        if name in dvo._SUB_OPCODE_FOR_NAME:
            return next(op for op in dvo.OPS if op.name == name)
        row = max(dvo._SUB_OPCODE_FOR_NAME.values()) + 1
        assert row < 0x20
        shas = {}
        for ver in ("v3", "v4"):
            uops = lower(spec, ver=ver)
            shas[ver] = DveOpSpec(name=name, opcode=row, uops=uops,
                                  rd1_en=rd1).sha(ver)
        op = dvo.DveOp(name, spec, subdim=False, uops_sha=shas)
        dvo._SUB_OPCODE_FOR_NAME[name] = row
        dvo.OPS.append(op)
        dvo.CUSTOM_DVE_SPECS[name] = spec
        return op

    t = Src0 + Src1 + C0
    op_z = reg(
        "LEAKY_Z_ANT",
        Spec(body=maxx(t, t * C2),
             reference=lambda in0, in1, s0, s1, imm2: np.maximum(
                 np.float32(in0) + np.float32(in1) + s0,
                 (np.float32(in0) + np.float32(in1) + s0) * imm2)),
        rd1=True)
    u = Src0 + C0
    op_zs = reg(
        "LEAKY_ZS_ANT",
        Spec(body=maxx(u, u * C2),
             reference=lambda in0, in1, s0, s1, imm2: np.maximum(
                 np.float32(in0) + s0, (np.float32(in0) + s0) * imm2)),
        rd1=False)
    return op_z, op_zs


OP_LEAKY_Z, OP_LEAKY_ZS = _register_dve_ops()


def _build_nc(n_cores=N_CORES, n=N):
    NB = n // 128          # node blocks (keys)
    OWN = n // n_cores     # own query rows per core
    OB = OWN // 128        # own row blocks
    NG = NB // GROUP       # slabs per attention pass
    nc = bacc.Bacc("TRN2", target_bir_lowering=False, debug=False,
                   num_devices=n_cores)

    # Pin every activation to the natural_log_exp_and_others table set
    # (it contains all four functions we use: Prelu, Exp, Ln, Copy).
    # Default set selection assigns Prelu/Exp and Ln to different sets,
    # causing a ~2.7us ACT_TABLE_LOAD+drain on every per-head reciprocal.
    import types
    import bass_rust as _bass_rust
    from concourse.hw_specs import get_activation_tables

    def _one_set_act_loads(self):
        has_activation = any(
            isinstance(i, mybir.InstActivation)
            for b in self.main_func.blocks
            for i in b.instructions)
        if not has_activation:
            return
        pin = "natural_log_exp_and_others"
        all_t = get_activation_tables(self.m.arch)
        pinned = all_t[pin]
        tables = [(name, fns if name == pin else fns - pinned)
                  for name, fns in all_t.items()]
        _bass_rust.insert_act_table_loads(self, tables)

    nc.insert_act_table_loads = types.MethodType(_one_set_act_loads, nc)

    # per-core external inputs (host-packed, see _pack_inputs)
    d_xT = nc.dram_tensor("xT", [128, KF * n], BF16, kind="ExternalInput")
    d_xo = nc.dram_tensor("xo", [128, KF * OWN], BF16, kind="ExternalInput")
    d_w64 = nc.dram_tensor("w64k", [128, KF * H * O], BF16, kind="ExternalInput")
    d_waA = nc.dram_tensor("waA", [128, KF * 2 * H], BF16, kind="ExternalInput")
    d_wo1 = nc.dram_tensor("wo1r", [128, KF * 128], BF16, kind="ExternalInput")
    d_nm = nc.dram_tensor("nmT", [128, NB * OWN], BF16, kind="ExternalInput")
    d_wot = nc.dram_tensor("wot", [128, KF * C], BF16, kind="ExternalInput")
    d_a2o = nc.dram_tensor("a2o", [128, C], F32, kind="ExternalInput")
    d_out = nc.dram_tensor("out", [OWN, C], F32, kind="ExternalOutput")

    with tile.TileContext(nc) as tc:
        with (
            tc.tile_pool(name="dram", bufs=1, space="DRAM") as dram,
            tc.tile_pool(name="const", bufs=1) as const,
            tc.tile_pool(name="work", bufs=2) as work,
            tc.tile_pool(name="small", bufs=2) as small,
            tc.tile_pool(name="psA", bufs=3, space="PSUM") as psA,
            tc.tile_pool(name="psH", bufs=2, space="PSUM") as psH,
            tc.tile_pool(name="psM", bufs=2, space="PSUM") as psM,
        ):
            # dummy activation first in program order: the pinned ACT table
            # set loads at t~0 (during input DMA) instead of stalling the
            # first real exp.
            warm = const.tile([1, 8], F32)
            nc.vector.memset(warm[:], 0.0)
            nc.scalar.activation(warm[:], warm[:], AF.Exp)

            # ---- load inputs (priority order: phase-S feeders first) ----
            waA = const.tile([128, KF * 2 * H], BF16)
            nc.sync.dma_start(out=waA, in_=d_waA[:])
            xo = const.tile([128, KF * OWN], BF16)
            for c in range(0, KF * OWN, 1024):
                nc.sync.dma_start(out=xo[:, c: c + 1024],
                                  in_=d_xo[:, c: c + 1024])
            # xT loads split: the first 1024 node-columns of each k-tile
            # feed s-chunks 0-1 and Wh blocks 0-7, so they go first;
            # the bulk follows.
            PFX = 1024
            xT = const.tile([128, KF * n], BF16)
            for k in range(KF):
                nc.sync.dma_start(out=xT[:, k * n: k * n + PFX],
                                  in_=d_xT[:, k * n: k * n + PFX])
            w64 = const.tile([128, KF * H * O], BF16)
            nc.sync.dma_start(out=w64, in_=d_w64[:])
            nm = const.tile([128, NB * OWN], BF16)
            wg = GROUP * OWN

            def load_nm(g):
                # 4 sub-chunks land on 4 DMA queues: ~4x less latency than
                # one 1MB transfer on a single ~25GB/s queue
                for c in range(4):
                    nc.sync.dma_start(
                        out=nm[:, g * wg + c * wg // 4: g * wg + (c + 1) * wg // 4],
                        in_=d_nm[:, g * wg + c * wg // 4: g * wg + (c + 1) * wg // 4])

            load_nm(0)
            # column-window-major order: s-chunk/Wh consumers need a column
            # window across ALL k-tiles, so land windows in consumption
            # order, with the nm slab each window's attention needs between
            for w0 in range(PFX, n, PFX):
                for k in range(KF):
                    nc.sync.dma_start(out=xT[:, k * n + w0: k * n + w0 + PFX],
                                      in_=d_xT[:, k * n + w0: k * n + w0 + PFX])
                load_nm(w0 // PFX)
            wo1 = const.tile([128, KF * 128], BF16)
            nc.sync.dma_start(out=wo1, in_=d_wo1[:])
            wot = const.tile([128, KF * C], BF16)
            nc.sync.dma_start(out=wot, in_=d_wot[:])
            a2o = const.tile([128, C], F32)
            nc.sync.dma_start(out=a2o, in_=d_a2o[:])

            ident = const.tile([128, 128], F32)
            make_identity(nc, ident[:])
            identb = const.tile([16, 16], BF16)
            make_identity(nc, identb[:])

            # all heads' Wh in one tile: [128, (h*NB + jb)*65 + {0..64|1}]
            whtA = const.tile([128, H * NB * 65], BF16)
            # ones columns for the softmax denominators, one strided memset
            nc.gpsimd.memset(
                whtA[:].rearrange("p (b w) -> p b w", w=65)[:, :, 64:65], 1.0)
            hT_all = const.tile([128, KF * OWN], BF16)

            # ---- phase S: s1/s2 rows via thin matmuls ----
            # s1 for own rows -> broadcast to all partitions per head
            ps1 = psM.tile([16, OWN], F32, tag="mm", name="ps1")
            for k in range(KF):
                nc.tensor.matmul(ps1[:], waA[:, k * 16: (k + 1) * 16],
                                 xo[:, k * OWN: (k + 1) * OWN],
                                 start=(k == 0), stop=(k == KF - 1))
            s1T = const.tile([16, OWN], BF16)
            nc.vector.tensor_copy(s1T[:], ps1[:])
            # broadcast row h to 128 partitions via a DRAM bounce; these
            # ride ScalarE's HWDGE queue - the sync queue is backed up with
            # bulk input loads at this point and would add ~30us of latency
            s1d = dram.tile([8, OWN], BF16)
            nc.scalar.dma_start(out=s1d[:], in_=s1T[8:16, :])
            s1b = [const.tile([128, OWN], BF16, tag=f"s1b{h}", name=f"s1b{h}")
                   for h in range(H)]
            for h in range(H):
                nc.scalar.dma_start(out=s1b[h][:],
                                    in_=s1d[h: h + 1, :].to_broadcast([128, OWN]))
            # s2 rows for all nodes; s2f split per slab-group and emitted
            # lazily (slab g only needs chunks/transposes 2g, 2g+1) so the
            # first attention slab is not gated on the whole sweep
            NGG = NB // GROUP
            s2fg = [const.tile([128, GROUP * 8], F32, tag=f"s2f{g}",
                               name=f"s2f{g}") for g in range(NGG)]

            def emit_s_chunk(ch):
                pss = psM.tile([16, 512], F32, tag="mm", name=f"s12_{ch}")
                for k in range(KF):
                    nc.tensor.matmul(
                        pss[:], waA[:, k * 16: (k + 1) * 16],
                        xT[:, k * n + ch * 512: k * n + ch * 512 + 512],
                        start=(k == 0), stop=(k == KF - 1))
                # copies on ScalarE: it idles through the head-0 window, and
                # keeping these off VectorE's in-order queue stops late xT
                # windows from head-of-line-blocking the first slabs
                s2c = small.tile([8, 512], BF16, tag="s2c", name=f"s2c{ch}")
                nc.scalar.activation(s2c[:], pss[0:8, :], AF.Copy)
                # transpose this chunk's 4 node-blocks into s2fg
                pst = psM.tile([128, 32], BF16, tag="s2t", name=f"s2t_{ch}",
                               bufs=1)
                for u in range(4):
                    nc.tensor.transpose(pst[:, u * 8: (u + 1) * 8],
                                        s2c[:, u * 128: (u + 1) * 128],
                                        identb[0:8, 0:8])
                jb = ch * 4
                nc.scalar.activation(
                    s2fg[jb // GROUP][:, (jb % GROUP) * 8: (jb % GROUP) * 8 + 32],
                    pst[:], AF.Copy)

            emit_s_chunk(0)
            emit_s_chunk(1)
            emit_s_chunk(2)
            emit_s_chunk(3)
            # (chunks 4-7 are emitted right after Wh blocks 0-7 below: the
            # s2fg feeds must all land early or V's slab customs starve
            # behind the PE's in-order Wh work)

            # ---- batched Wh: one node-block, ALL 8 heads, one PSUM bank ----
            def emit_wh_block(jb):
                ps = psA.tile([128, H * O], F32, tag="whp", name=f"whp_{jb}")
                for k in range(KF):
                    nc.tensor.matmul(
                        ps[:], xT[:, k * n + jb * 128: k * n + jb * 128 + 128],
                        w64[:, k * H * O: (k + 1) * H * O],
                        start=(k == 0), stop=(k == KF - 1))
                dst = (whtA[:].rearrange("p (h b w) -> p h b w", h=H, w=65)
                       [:, :, jb, 0:O])
                src = ps[:].rearrange("p (h w) -> p h w", w=O)
                # first blocks cast on V (idle at ramp), rest on ScalarE
                if jb < 8:
                    nc.vector.tensor_copy(dst, src)
                else:
                    nc.scalar.activation(dst, src, AF.Copy)

            for jb in range(8):
                emit_wh_block(jb)
            for ch in range(4, 8):
                emit_s_chunk(ch)
            for jb in range(8, 16):
                emit_wh_block(jb)

            def whA(h):
                def lhsT(jb):
                    return whtA[:, (h * NB + jb) * 65: (h * NB + jb) * 65 + 65]
                return lhsT

            # ---- attention slab helper (hidden + output layers) ----
            def attention(s2col, s1bt, lhsT, m_rows, psacc, tagp,
                          pre_slab=None, post_slab=None, halves=False,
                          s_blocks=S_BLOCKS, p_blocks=P_BLOCKS, lhsT2=None):
                for g in range(NG):
                    if pre_slab is not None:
                        pre_slab(g)
                    us = work.tile([128, GROUP * OWN], BF16, tag="p",
                                   name=f"u{tagp}_{g}")
                    # os bufs=4: head-0's matmuls lag the V/S pace while PE
                    # (in-order queue) interleaves Wh-block emission; extra
                    # os buffers let V run ahead into the next head instead
                    # of stalling on the slab-matmul release
                    os_ = work.tile([128, GROUP * OWN], BF16, tag="o",
                                    name=f"o{tagp}_{g}", bufs=4)
                    # halves: process the slab as two 4-block pieces so the
                    # first matmuls (and the last exp) are earlier -
                    # shortens the latency-bound head-0/output tail
                    chunks = ([(0, GROUP // 2), (GROUP // 2, GROUP)]
                              if halves else [(0, GROUP)])
                    for (q0, q1) in chunks:
                        for q in range(q0, q1):
                            jb = g * GROUP + q
                            dst = us[:, q * OWN: (q + 1) * OWN]
                            nmb = nm[:, jb * OWN: (jb + 1) * OWN]
                            r = q - q0
                            if r < s_blocks:
                                # S-routed block: z-base TT on V, +s2 fused
                                # into the Prelu bias on ScalarE
                                zt = small.tile([128, OWN], BF16, tag="zb",
                                                name=f"zb{tagp}_{g}_{q}")
                                nc.vector.tensor_tensor(zt[:], nmb, s1bt[:],
                                                        ADD)
                                nc.scalar.activation(dst, zt[:], AF.Prelu,
                                                     bias=s2col(jb),
                                                     alpha=ALPHA)
                            elif r < s_blocks + p_blocks:
                                # Pool-routed block: z-base TT on GpSimd
                                # (only TT/copy/memset are legal on Pool),
                                # +s2 fused into the Prelu bias on ScalarE
                                ztp = small.tile([128, OWN], BF16, tag="zp",
                                                 name=f"zp{tagp}_{g}_{q}")
                                nc.gpsimd.tensor_tensor(ztp[:], nmb,
                                                        s1bt[:], ADD)
                                nc.scalar.activation(dst, ztp[:], AF.Prelu,
                                                     bias=s2col(jb),
                                                     alpha=ALPHA)
                            else:
                                nc.vector._custom_dve(
                                    OP_LEAKY_Z, out=dst, in0=nmb,
                                    in1=s1bt[:], s0=s2col(jb), imm2=ALPHA)
                        nc.scalar.activation(os_[:, q0 * OWN: q1 * OWN],
                                             us[:, q0 * OWN: q1 * OWN], AF.Exp)
                        for q in range(q0, q1):
                            jb = g * GROUP + q
                            mms = [lhsT(jb)] + ([lhsT2(jb)] if lhsT2 else [])
                            for mi, l in enumerate(mms):
                                nc.tensor.matmul(
                                    psacc[0: m_rows, :], l,
                                    os_[:, q * OWN: (q + 1) * OWN],
                                    start=(jb == 0 and mi == 0),
                                    stop=(jb == NB - 1 and mi == len(mms) - 1))
                    if post_slab is not None:
                        post_slab(g)

            def emit_who_half(tag, cs, prng=None):
                # prng: (p0, p1) partition half for a single-head (K=64)
                # contribution; None = full 128-partition c-blocks
                p0, p1 = prng if prng else (0, 128)
                who = const.tile([128, OB * 17], F32, name=f"whoown{tag}")
                for ib in range(OB):
                    pw = psM.tile([128, OWN], F32, tag="mm",
                                  name=f"pw{tag}{ib}")
                    for ci, c in enumerate(cs):
                        nc.tensor.matmul(
                            pw[:, 0:C],
                            hT_all[p0:p1, c * OWN + ib * 128: c * OWN + ib * 128 + 128],
                            wot[p0:p1, c * C: (c + 1) * C],
                            start=(ci == 0), stop=(ci == len(cs) - 1))
                    nc.vector.tensor_copy(who[:, ib * 17: ib * 17 + C],
                                          pw[:, 0:C])
                    tmp = small.tile([128, C], F32, tag="s2tmp",
                                     name=f"s2o{tag}{ib}")
                    nc.vector.scalar_tensor_tensor(
                        tmp[:], pw[:, 0:C], 1.0, a2o[:], MULT, MULT,
                        accum_out=who[:, ib * 17 + 16: ib * 17 + 17])
                return who

            def emit_gather_start(tag, src_ap, width):
                # ci out-DMA on the Pool queue, which carries ONLY DMA/CC
                # work (no arith -> no SWDGE<->arith ucode lib reloads);
                # the sync queue is blocked mid-phase by gather reads.
                ci = dram.tile([128, width], F32, name=f"cci{tag}")
                co = dram.tile([n_cores * 128, width], F32,
                               addr_space="Shared" if n_cores > 1 else "Local",
                               name=f"cco{tag}")
                nc.gpsimd.dma_start(out=ci[:], in_=src_ap)
                if n_cores > 1:
                    nc.gpsimd.collective_compute(
                        "AllGather", mybir.AluOpType.bypass,
                        replica_groups=[list(range(n_cores))],
                        ins=[ci.opt()], outs=[co.opt()])
                else:
                    nc.sync.dma_start(out=co[:], in_=ci[:])
                return co

            def emit_gather_read(tag, co, out_w, nsplit=1):
                ga = const.tile([128, out_w], F32, name=f"ga{tag}")
                src = co[:].rearrange("(g p) f -> p g f", p=128)
                step = n_cores // nsplit
                for qi in range(nsplit):
                    nc.sync.dma_start(
                        out=ga[:].rearrange("p (g f) -> p g f", g=n_cores)
                        [:, qi * step: (qi + 1) * step, :],
                        in_=src[:, qi * step: (qi + 1) * step, :])
                return ga

            # ---- phase B: hidden attention ----
            def finalize_head(h, ph):
                # 1/den = exp(-ln(den)) on ScalarE, straight from PSUM
                lnr = small.tile([65, OWN], F32, tag="lnr", name=f"lnr{h}", bufs=1)
                nc.scalar.activation(lnr[64:65, :], ph[64:65, :], AF.Ln)
                nc.scalar.activation(lnr[64:65, :], lnr[64:65, :], AF.Exp,
                                     scale=-1.0)
                rd = dram.tile([1, OWN], F32, name=f"rd{h}")
                # producer-local HWDGE triggers for ALL heads: the rd
                # trigger directly follows lnr's Exp on ScalarE (wait always
                # satisfied), while the sync queue is blocked mid-phase by
                # gather reads waiting on collectives - a sync-queued rd
                # here stalled the finalize (and V behind it) ~17us.
                dq = nc.scalar
                dq.dma_start(out=rd[:], in_=lnr[64:65, :])
                rb = small.tile([64, OWN], F32, tag="rb", name=f"rb{h}")
                dq.dma_start(out=rb[:],
                             in_=rd[0:1, :].to_broadcast([64, OWN]))
                tn = small.tile([64, OWN], F32, tag="tn", name=f"tn{h}", bufs=1)
                nc.vector.tensor_tensor(tn[:], ph[0:64, :], rb[:], MULT)
                m0 = small.tile([64, OWN], F32, tag="rb", name=f"m0{h}")
                nc.vector.tensor_scalar(m0[:], tn[:], 0.0, None,
                                        mybir.AluOpType.min)
                g_ = small.tile([64, OWN], F32, tag="g", name=f"g{h}", bufs=1)
                nc.scalar.activation(g_[:], m0[:], AF.Exp)
                slot = hT_all[(h % 2) * 64: (h % 2) * 64 + 64,
                              (h // 2) * OWN: (h // 2) * OWN + OWN]
                nc.vector.scalar_tensor_tensor(slot, g_[:], -1.0, tn[:], ADD, MAX)

            prev = [None]
            gaA_ref = [None]
            coC_ref = [None]
            coE_ref = [None]
            whoAC = const.tile([128, NB * 17], F32)
            whoACE = const.tile([128, NB * 17], F32)
            s2oX = const.tile([128, NB], F32)
            s2oY = const.tile([128, NB], F32)
            for h in range(H):
                ph = psH.tile([65, OWN], F32, tag="ph", name=f"ph{h}")

                def pre_slab(g, h=h, ph=ph):
                    if g == (1 if NG > 1 else 0) and prev[0] is not None:
                        # previous head's normalize/elu, off the critical path
                        finalize_head(h - 1, prev[0])
                    if h == 4 and g == 2:
                        # heads 0-3 finalized: gather their Wh_o contribution
                        # while heads 4-7 compute. Emitted mid-head so the
                        # pw matmuls/copies land mid-queue, not behind the
                        # whole head's work.
                        whoA = emit_who_half("A", [0, 1])
                        coA = emit_gather_start("A", whoA[:], OB * 17)
                        gaA_ref[0] = emit_gather_read("A", coA, NB * 17)
                    if h == 6 and g == 2:
                        # heads 4-5 finalized: start their gather during
                        # heads 6-7
                        whoC = emit_who_half("C", [2])
                        coC_ref[0] = emit_gather_start("C", whoC[:], OB * 17)
                    if h == 7 and g == 2:
                        # head 6 finalized (at g==1): its half of c-block 3
                        # via K=64 matmuls; gather during head 7's tail
                        whoE = emit_who_half("E", [3], prng=(0, 64))
                        coE_ref[0] = emit_gather_start("E", whoE[:], OB * 17)
                    if h == 7 and g == 3:
                        # C completed during head 6/7: read + combine with A
                        # on VectorE (GpSimd arith here would force ucode
                        # lib swaps around the collective triggers)
                        gaC = emit_gather_read("C", coC_ref[0], NB * 17)
                        gaA = gaA_ref[0]
                        nc.vector.tensor_tensor(whoAC[:], gaA[:], gaC[:], ADD)
                        nc.vector.tensor_tensor(
                            s2oX[:].rearrange("p (b w) -> p b w", w=1),
                            gaA[:].rearrange("p (b w) -> p b w", w=17)[:, :, 16:17],
                            gaC[:].rearrange("p (b w) -> p b w", w=17)[:, :, 16:17],
                            ADD)

                def post_slab(g, h=h):
                    # head 0: remaining Wh blocks two slabs ahead, after the
                    # slab's own work so the casts/matmuls don't head-of-
                    # line-block the slab's exps and attention matmuls
                    if h == 0:
                        for j in range((g + 2) * GROUP,
                                       min((g + 3) * GROUP, NB)):
                            emit_wh_block(j)

                attention(lambda jb, h=h: s2fg[jb // GROUP][:, (jb % GROUP) * 8 + h: (jb % GROUP) * 8 + h + 1],
                          s1b[h], whA(h), 65, ph, f"h{h}",
                          pre_slab=pre_slab, post_slab=post_slab,
                          halves=(h in (0, H - 1)))
                prev[0] = ph
            finalize_head(H - 1, prev[0])

            # ---- phase C tail: only head-7's half of c-block 3 (~35KB)
            # plus the tiny s2o column remain to gather. ----
            # s1ob depends only on hT_all: emit before the final collective
            s1ob = const.tile([128, OWN], BF16)
            ps1o = psM.tile([128, OWN], F32, tag="mm", name="ps1o")
            for c in range(KF):
                nc.tensor.matmul(ps1o[:], wo1[:, c * 128: (c + 1) * 128],
                                 hT_all[:, c * OWN: (c + 1) * OWN],
                                 start=(c == 0), stop=(c == KF - 1))
            nc.vector.tensor_copy(s1ob[:], ps1o[:])

            # who17 = bf16(whoACE) with the ones column (denominators);
            # who2 = bf16(gaF) with a ZERO 17th column. The A+C+E / F halves
            # are summed by the PE itself: two accumulating matmuls per
            # j-block, so no f32 assembly pass sits on the critical tail.
            who17 = const.tile([128, NB * 17], BF16)
            nc.vector.memset(
                who17[:].rearrange("p (b w) -> p b w", w=17)[:, :, 16:17], 1.0)
            who2 = const.tile([128, NB * 17], BF16)
            nc.vector.memset(
                who2[:].rearrange("p (b w) -> p b w", w=17)[:, :, 16:17], 0.0)

            whoF = emit_who_half("F", [3], prng=(64, 128))
            # collectives dispatched FIRST (tiny s2o column, then the F
            # half): nothing may queue ahead of their ci DMAs/doorbells
            coS = emit_gather_start(
                "S", whoF[:].rearrange("p (b w) -> p b w", w=17)[:, :, 16:17],
                OB)
            coF = emit_gather_start("F", whoF[:], OB * 17)
            # E-half readback + A+C+E combines on V (E completed during
            # head 7); who17 cast is ready before the F data lands
            gaE = emit_gather_read("E", coE_ref[0], NB * 17)
            nc.vector.tensor_tensor(whoACE[:], whoAC[:], gaE[:], ADD)
            nc.vector.tensor_tensor(
                s2oY[:].rearrange("p (b w) -> p b w", w=1),
                s2oX[:].rearrange("p (b w) -> p b w", w=1),
                gaE[:].rearrange("p (b w) -> p b w", w=17)[:, :, 16:17],
                ADD)
            nc.vector.tensor_copy(
                who17[:].rearrange("p (b w) -> p b w", w=17)[:, :, 0:C],
                whoACE[:].rearrange("p (b w) -> p b w", w=17)[:, :, 0:C])
            s2obF = emit_gather_read("S", coS, NB)
            s2oall = const.tile([128, NB], F32)
            nc.vector.tensor_tensor(
                s2oall[:].rearrange("p (b w) -> p b w", w=1),
                s2oY[:].rearrange("p (b w) -> p b w", w=1),
                s2obF[:].rearrange("p (b w) -> p b w", w=1),
                ADD)
            gaF = emit_gather_read("F", coF, NB * 17, nsplit=4)
            # F-half cast on ScalarE: lands between the output layer's exps
            nc.scalar.activation(
                who2[:].rearrange("p (b w) -> p b w", w=17)[:, :, 0:C],
                gaF[:].rearrange("p (b w) -> p b w", w=17)[:, :, 0:C],
                AF.Copy)

            # ---- phase D: output attention ----
            po = psM.tile([128, OWN], F32, tag="mm", name="po")

            attention(lambda jb: s2oall[:, jb: jb + 1],
                      s1ob, lambda jb: who17[:, jb * 17: jb * 17 + 17],
                      17, po, "o", halves=True, s_blocks=1, p_blocks=0,
                      lhsT2=lambda jb: who2[:, jb * 17: jb * 17 + 17])

            # ---- phase E: transpose, normalize, elu, log_softmax, store ----
            osb = const.tile([17, OWN], F32)
            nc.scalar.activation(osb[:], po[0:17, :], AF.Copy)
            ptr = psM.tile([128, OB * 17], F32, tag="mm", name="ptr")
            for tt in range(OB):
                nc.tensor.transpose(ptr[:, tt * 17: tt * 17 + 17],
                                    osb[0:17, tt * 128: (tt + 1) * 128],
                                    ident[0:17, 0:17])
            es = const.tile([128, OB * 17], F32)
            nc.vector.tensor_copy(es[:], ptr[:])
            rec4 = const.tile([128, OB], F32)
            nc.vector.reciprocal(
                rec4[:], es[:].rearrange("p (b w) -> p b w", w=17)[:, :, 16:17])
            # batched normalize / elu / log_softmax over all OB row-blocks
            t1 = const.tile([128, OB * C], F32)
            nc.vector.tensor_tensor(
                t1[:].rearrange("p (b w) -> p b w", w=C),
                es[:].rearrange("p (b w) -> p b w", w=17)[:, :, 0:C],
                rec4[:].rearrange("p (b w) -> p b w", w=1)
                .to_broadcast([128, OB, C]),
                MULT)
            m1 = const.tile([128, OB * C], F32)
            nc.vector.tensor_scalar(m1[:], t1[:], 0.0, None,
                                    mybir.AluOpType.min)
            g1 = const.tile([128, OB * C], F32)
            nc.scalar.activation(g1[:], m1[:], AF.Exp)
            e1all = const.tile([128, OB * C], F32)
            nc.vector.scalar_tensor_tensor(e1all[:], g1[:], -1.0, t1[:],
                                           ADD, MAX)
            sall = const.tile([128, OB], F32)
            final = const.tile([128, OB * C], F32)
            for tt in range(OB):
                ex = small.tile([128, C], F32, tag="ex", name=f"ex{tt}")
                nc.scalar.activation(ex[:], e1all[:, tt * C: (tt + 1) * C],
                                     AF.Exp, accum_out=sall[:, tt: tt + 1])
            lns = const.tile([128, OB], F32)
            nc.scalar.activation(lns[:], sall[:], AF.Ln)
            nc.vector.tensor_tensor(
                final[:].rearrange("p (b w) -> p b w", w=C),
                e1all[:].rearrange("p (b w) -> p b w", w=C),
                lns[:].rearrange("p (b w) -> p b w", w=1)
                .to_broadcast([128, OB, C]),
                mybir.AluOpType.subtract)
            nc.sync.dma_start(
                out=d_out[:].rearrange("(b p) c -> p b c", p=128),
                in_=final[:])

    nc.compile()
    return nc


def _pack_inputs(x, adj, W, a, W_out, a_out, n_cores=N_CORES):
    """Host-side shard + layout packing. Returns list of per-core in_maps."""
    n, f_in = x.shape
    OWN = n // n_cores
    NB = n // 128
    xf = np.asarray(x, np.float32)
    adj = np.asarray(adj)
    Wf = np.asarray(W, np.float32)
    af = np.asarray(a, np.float32)
    Wof = np.asarray(W_out, np.float32)
    aof = np.asarray(a_out, np.float32)

    # xT[p, k*n + m] = x[m, 128k+p]
    xT = xf.T.reshape(KF, 128, n).transpose(1, 0, 2).reshape(128, KF * n)
    xT = xT.astype(BF)
    # k-major, head-minor weight layout: w64k[p, (k*H + h)*O + o]
    w64k = (Wf.reshape(H, KF, 128, O).transpose(2, 1, 0, 3)
            .reshape(128, KF * H * O).astype(BF))
    wa1 = np.einsum("hfo,ho->hf", Wf, af[:, :O])  # [H, F]
    wa2 = np.einsum("hfo,ho->hf", Wf, af[:, O:])
    # waA[p, k*16 + m]: m<8 -> wa2[m], else wa1[m-8]
    waA = np.concatenate([wa2, wa1], axis=0)  # [16, F]
    waA = waA.T.reshape(KF, 128, 16).transpose(1, 0, 2).reshape(128, KF * 16)
    waA = waA.astype(BF)
    wo1 = Wof @ aof[:C]  # [F]
    wo1r = np.broadcast_to(
        wo1.reshape(KF, 128).T[:, :, None], (128, KF, 128)
    ).reshape(128, KF * 128).astype(BF)
    wot = (Wof.reshape(KF, 128, C).transpose(1, 0, 2)
           .reshape(128, KF * C).astype(BF))
    a2o = np.broadcast_to(aof[C:], (128, C)).astype(np.float32).copy()

    in_maps = []
    for core in range(n_cores):
        rows = slice(core * OWN, (core + 1) * OWN)
        xo = (xf[rows].T.reshape(KF, 128, OWN).transpose(1, 0, 2)
              .reshape(128, KF * OWN).astype(BF))
        nmT = np.where(adj[rows].T > 0, np.float32(0), np.float32(NEG))
        nmT = (nmT.reshape(NB, 128, OWN).transpose(1, 0, 2)
               .reshape(128, NB * OWN).astype(BF))
        in_maps.append({
            "xT": xT, "xo": xo, "w64k": w64k, "waA": waA, "wo1r": wo1r,
            "nmT": nmT, "wot": wot, "a2o": a2o,
        })
    return in_maps


_NC_CACHE = {}


def _get_nc(n_cores=N_CORES, n=N):
    key = (n_cores, n)
    if key not in _NC_CACHE:
        _NC_CACHE[key] = _build_nc(n_cores, n)
    return _NC_CACHE[key]


def kernel(x, adj, W, a, W_out, a_out):
    nc = _get_nc()
    in_maps = _pack_inputs(x, adj, W, a, W_out, a_out)
    res = run_bass_kernel_spmd(nc, in_maps, list(range(N_CORES)))
    out = np.concatenate([res.results[c]["out"] for c in range(N_CORES)], axis=0)
    return out.astype(np.float32)


# revision 30
# speedup vs baseline: 1.0180x; 1.0180x over previous
"""GAT (graph attention network) Bass kernel for 8 trn2 NeuronCores.

Strategy (row-sharding): core k owns query rows [k*512, (k+1)*512).
 - Every core computes Wh = x @ W[h] for ALL nodes (replicated, cheap on PE)
   in [node-on-partition, feature] layout; s1 = x@(W a1) and s2 = x@(W a2)
   come from one thin matmul against a packed [wa1|wa2] weight block.
   Wh is batched: ALL 8 heads accumulate into one PSUM bank per node-block
   (k-major weight layout), one f32->bf16 cast per block.
 - Hidden attention per head, transposed layout [keys j on partitions,
   own rows i free]: us = leaky(nm + s1[i] + s2[j]) is ONE custom DVE op
   per j-block (4 ALU stages: add, add, mul-alpha, max), registered into
   the per-NEFF DVE table at import. This removes the separate z-base
   tensor_tensor, the per-block +s2 tensor_scalar, and the ScalarE Prelu
   entirely -- ScalarE does exp only. out_head.T = [Wh|1].T @ P
   accumulated in PSUM gives both att@Wh and softmax denominators.
 - Per-head normalize: 1/den via exp(-ln(den)) on ScalarE (reads PSUM
   directly), then mult + elu -> h kept transposed as lhsT for
   Wh_o = h @ W_out. The Wh_o partial-sum AllGather is split 4 ways:
   c-blocks {0,1} at head 4, {2} at head 6, head-6's half of c-block 3
   (K=64 matmul) during head 7, so only head-7's half (~35KB) plus a tiny
   s2o-column gather sit on the critical tail.
 - Output attention layer runs the same fused-op way (s0 = gathered s2o),
   followed by elu + log_softmax.
"""

import sys

sys.path.insert(0, "/opt/trn_rl_repo")

import numpy as np
import ml_dtypes

import concourse.bass as bass
import concourse.bacc as bacc
import concourse.tile as tile
from concourse import mybir
from concourse.bass_utils import run_bass_kernel_spmd
from concourse.masks import make_identity

F32 = mybir.dt.float32
BF16 = mybir.dt.bfloat16
BF = ml_dtypes.bfloat16
ADD = mybir.AluOpType.add
MULT = mybir.AluOpType.mult
MAX = mybir.AluOpType.max
AF = mybir.ActivationFunctionType

# problem shape (hardcoded per spec)
N = 4096
F_IN = 512
O = 64
H = 8
C = 16
N_CORES = 8
NEG = -100.0  # additive mask offset; exp(leaky(-100+e)) <= ~1e-8
ALPHA = 0.2

# knobs
GROUP = 8          # j-blocks per activation slab
S_BLOCKS = 1       # per-slab blocks routed {TT on V + bias-Prelu on S}
                   # instead of the fused custom op (S/V balance fine-tune)
P_BLOCKS = 0       # per-slab blocks with the z-base TT on GpSimd instead of V
KF = F_IN // 128   # f_in k-tiles


# ---- custom DVE ops: fused us = max(t, alpha*t), t = in0 (+ in1) + s0 ----
def _register_dve_ops():
    from concourse import dve_ops as dvo
    from concourse.dve_spec import Spec, Src0, Src1, C0, C2, maxx, lower
    from concourse.dve_uop import DveOpSpec

    def reg(name, spec, rd1):
        if name in dvo._SUB_OPCODE_FOR_NAME:
            return next(op for op in dvo.OPS if op.name == name)
        row = max(dvo._SUB_OPCODE_FOR_NAME.values()) + 1
        assert row < 0x20
        shas = {}
        for ver in ("v3", "v4"):
            uops = lower(spec, ver=ver)
            shas[ver] = DveOpSpec(name=name, opcode=row, uops=uops,
                                  rd1_en=rd1).sha(ver)
        op = dvo.DveOp(name, spec, subdim=False, uops_sha=shas)
        dvo._SUB_OPCODE_FOR_NAME[name] = row
        dvo.OPS.append(op)
        dvo.CUSTOM_DVE_SPECS[name] = spec
        return op

    t = Src0 + Src1 + C0
    op_z = reg(
        "LEAKY_Z_ANT",
        Spec(body=maxx(t, t * C2),
             reference=lambda in0, in1, s0, s1, imm2: np.maximum(
                 np.float32(in0) + np.float32(in1) + s0,
                 (np.float32(in0) + np.float32(in1) + s0) * imm2)),
        rd1=True)
    u = Src0 + C0
    op_zs = reg(
        "LEAKY_ZS_ANT",
        Spec(body=maxx(u, u * C2),
             reference=lambda in0, in1, s0, s1, imm2: np.maximum(
                 np.float32(in0) + s0, (np.float32(in0) + s0) * imm2)),
        rd1=False)
    return op_z, op_zs


OP_LEAKY_Z, OP_LEAKY_ZS = _register_dve_ops()


def _build_nc(n_cores=N_CORES, n=N):
    NB = n // 128          # node blocks (keys)
    OWN = n // n_cores     # own query rows per core
    OB = OWN // 128        # own row blocks
    NG = NB // GROUP       # slabs per attention pass
    nc = bacc.Bacc("TRN2", target_bir_lowering=False, debug=False,
                   num_devices=n_cores)

    # Pin every activation to the natural_log_exp_and_others table set
    # (it contains all four functions we use: Prelu, Exp, Ln, Copy).
    # Default set selection assigns Prelu/Exp and Ln to different sets,
    # causing a ~2.7us ACT_TABLE_LOAD+drain on every per-head reciprocal.
    import types
    import bass_rust as _bass_rust
    from concourse.hw_specs import get_activation_tables

    def _one_set_act_loads(self):
        has_activation = any(
            isinstance(i, mybir.InstActivation)
            for b in self.main_func.blocks
            for i in b.instructions)
        if not has_activation:
            return
        pin = "natural_log_exp_and_others"
        all_t = get_activation_tables(self.m.arch)
        pinned = all_t[pin]
        tables = [(name, fns if name == pin else fns - pinned)
                  for name, fns in all_t.items()]
        _bass_rust.insert_act_table_loads(self, tables)

    nc.insert_act_table_loads = types.MethodType(_one_set_act_loads, nc)

    # per-core external inputs (host-packed, see _pack_inputs)
    d_xT = nc.dram_tensor("xT", [128, KF * n], BF16, kind="ExternalInput")
    d_xo = nc.dram_tensor("xo", [128, KF * OWN], BF16, kind="ExternalInput")
    d_w64 = nc.dram_tensor("w64k", [128, KF * H * O], BF16, kind="ExternalInput")
    d_waA = nc.dram_tensor("waA", [128, KF * 2 * H], BF16, kind="ExternalInput")
    d_wo1 = nc.dram_tensor("wo1r", [128, KF * 128], BF16, kind="ExternalInput")
    d_nm = nc.dram_tensor("nmT", [128, NB * OWN], BF16, kind="ExternalInput")
    d_wot = nc.dram_tensor("wot", [128, KF * C], BF16, kind="ExternalInput")
    d_a2o = nc.dram_tensor("a2o", [128, C], F32, kind="ExternalInput")
    d_out = nc.dram_tensor("out", [OWN, C], F32, kind="ExternalOutput")

    with tile.TileContext(nc) as tc:
        with (
            tc.tile_pool(name="dram", bufs=1, space="DRAM") as dram,
            tc.tile_pool(name="const", bufs=1) as const,
            tc.tile_pool(name="work", bufs=2) as work,
            tc.tile_pool(name="small", bufs=2) as small,
            tc.tile_pool(name="psA", bufs=3, space="PSUM") as psA,
            tc.tile_pool(name="psH", bufs=2, space="PSUM") as psH,
            tc.tile_pool(name="psM", bufs=2, space="PSUM") as psM,
        ):
            # dummy activation first in program order: the pinned ACT table
            # set loads at t~0 (during input DMA) instead of stalling the
            # first real exp.
            warm = const.tile([1, 8], F32)
            nc.vector.memset(warm[:], 0.0)
            nc.scalar.activation(warm[:], warm[:], AF.Exp)

            # ---- load inputs (priority order: phase-S feeders first) ----
            waA = const.tile([128, KF * 2 * H], BF16)
            nc.sync.dma_start(out=waA, in_=d_waA[:])
            xo = const.tile([128, KF * OWN], BF16)
            for c in range(0, KF * OWN, 1024):
                nc.sync.dma_start(out=xo[:, c: c + 1024],
                                  in_=d_xo[:, c: c + 1024])
            # xT loads split: the first 1024 node-columns of each k-tile
            # feed s-chunks 0-1 and Wh blocks 0-7, so they go first;
            # the bulk follows.
            PFX = 1024
            xT = const.tile([128, KF * n], BF16)
            for k in range(KF):
                nc.sync.dma_start(out=xT[:, k * n: k * n + PFX],
                                  in_=d_xT[:, k * n: k * n + PFX])
            w64 = const.tile([128, KF * H * O], BF16)
            nc.sync.dma_start(out=w64, in_=d_w64[:])
            nm = const.tile([128, NB * OWN], BF16)
            wg = GROUP * OWN

            def load_nm(g):
                # 4 sub-chunks land on 4 DMA queues: ~4x less latency than
                # one 1MB transfer on a single ~25GB/s queue
                for c in range(4):
                    nc.sync.dma_start(
                        out=nm[:, g * wg + c * wg // 4: g * wg + (c + 1) * wg // 4],
                        in_=d_nm[:, g * wg + c * wg // 4: g * wg + (c + 1) * wg // 4])

            load_nm(0)
            # column-window-major order: s-chunk/Wh consumers need a column
            # window across ALL k-tiles, so land windows in consumption
            # order, with the nm slab each window's attention needs between
            for w0 in range(PFX, n, PFX):
                for k in range(KF):
                    nc.sync.dma_start(out=xT[:, k * n + w0: k * n + w0 + PFX],
                                      in_=d_xT[:, k * n + w0: k * n + w0 + PFX])
                load_nm(w0 // PFX)
            wo1 = const.tile([128, KF * 128], BF16)
            nc.sync.dma_start(out=wo1, in_=d_wo1[:])
            wot = const.tile([128, KF * C], BF16)
            nc.sync.dma_start(out=wot, in_=d_wot[:])
            a2o = const.tile([128, C], F32)
            nc.sync.dma_start(out=a2o, in_=d_a2o[:])

            ident = const.tile([128, 128], F32)
            make_identity(nc, ident[:])
            identb = const.tile([16, 16], BF16)
            make_identity(nc, identb[:])

            # all heads' Wh in one tile: [128, (h*NB + jb)*65 + {0..64|1}]
            whtA = const.tile([128, H * NB * 65], BF16)
            # ones columns for the softmax denominators, one strided memset
            nc.gpsimd.memset(
                whtA[:].rearrange("p (b w) -> p b w", w=65)[:, :, 64:65], 1.0)
            hT_all = const.tile([128, KF * OWN], BF16)

            # ---- phase S: s1/s2 rows via thin matmuls ----
            # s1 for own rows -> broadcast to all partitions per head
            ps1 = psM.tile([16, OWN], F32, tag="mm", name="ps1")
            for k in range(KF):
                nc.tensor.matmul(ps1[:], waA[:, k * 16: (k + 1) * 16],
                                 xo[:, k * OWN: (k + 1) * OWN],
                                 start=(k == 0), stop=(k == KF - 1))
            s1T = const.tile([16, OWN], BF16)
            nc.vector.tensor_copy(s1T[:], ps1[:])
            # broadcast row h to 128 partitions via a DRAM bounce; these
            # ride ScalarE's HWDGE queue - the sync queue is backed up with
            # bulk input loads at this point and would add ~30us of latency
            s1d = dram.tile([8, OWN], BF16)
            nc.scalar.dma_start(out=s1d[:], in_=s1T[8:16, :])
            s1b = [const.tile([128, OWN], BF16, tag=f"s1b{h}", name=f"s1b{h}")
                   for h in range(H)]
            for h in range(H):
                nc.scalar.dma_start(out=s1b[h][:],
                                    in_=s1d[h: h + 1, :].to_broadcast([128, OWN]))
            # s2 rows for all nodes; s2f split per slab-group and emitted
            # lazily (slab g only needs chunks/transposes 2g, 2g+1) so the
            # first attention slab is not gated on the whole sweep
            NGG = NB // GROUP
            s2fg = [const.tile([128, GROUP * 8], F32, tag=f"s2f{g}",
                               name=f"s2f{g}") for g in range(NGG)]

            def emit_s_chunk(ch):
                pss = psM.tile([16, 512], F32, tag="mm", name=f"s12_{ch}")
                for k in range(KF):
                    nc.tensor.matmul(
                        pss[:], waA[:, k * 16: (k + 1) * 16],
                        xT[:, k * n + ch * 512: k * n + ch * 512 + 512],
                        start=(k == 0), stop=(k == KF - 1))
                # copies on ScalarE: it idles through the head-0 window, and
                # keeping these off VectorE's in-order queue stops late xT
                # windows from head-of-line-blocking the first slabs
                s2c = small.tile([8, 512], BF16, tag="s2c", name=f"s2c{ch}")
                nc.scalar.activation(s2c[:], pss[0:8, :], AF.Copy)
                # transpose this chunk's 4 node-blocks into s2fg
                pst = psM.tile([128, 32], BF16, tag="s2t", name=f"s2t_{ch}",
                               bufs=1)
                for u in range(4):
                    nc.tensor.transpose(pst[:, u * 8: (u + 1) * 8],
                                        s2c[:, u * 128: (u + 1) * 128],
                                        identb[0:8, 0:8])
                jb = ch * 4
                nc.scalar.activation(
                    s2fg[jb // GROUP][:, (jb % GROUP) * 8: (jb % GROUP) * 8 + 32],
                    pst[:], AF.Copy)

            emit_s_chunk(0)
            emit_s_chunk(1)
            emit_s_chunk(2)
            emit_s_chunk(3)
            # (chunks 4-7 are emitted right after Wh blocks 0-7 below: the
            # s2fg feeds must all land early or V's slab customs starve
            # behind the PE's in-order Wh work)

            # ---- batched Wh: one node-block, ALL 8 heads, one PSUM bank ----
            def emit_wh_block(jb):
                ps = psA.tile([128, H * O], F32, tag="whp", name=f"whp_{jb}")
                for k in range(KF):
                    nc.tensor.matmul(
                        ps[:], xT[:, k * n + jb * 128: k * n + jb * 128 + 128],
                        w64[:, k * H * O: (k + 1) * H * O],
                        start=(k == 0), stop=(k == KF - 1))
                dst = (whtA[:].rearrange("p (h b w) -> p h b w", h=H, w=65)
                       [:, :, jb, 0:O])
                src = ps[:].rearrange("p (h w) -> p h w", w=O)
                # first blocks cast on V (idle at ramp), rest on ScalarE
                if jb < 8:
                    nc.vector.tensor_copy(dst, src)
                else:
                    nc.scalar.activation(dst, src, AF.Copy)

            for jb in range(8):
                emit_wh_block(jb)
            for ch in range(4, 8):
                emit_s_chunk(ch)
            for jb in range(8, 16):
                emit_wh_block(jb)

            def whA(h):
                def lhsT(jb):
                    return whtA[:, (h * NB + jb) * 65: (h * NB + jb) * 65 + 65]
                return lhsT

            # ---- attention slab helper (hidden + output layers) ----
            def attention(s2col, s1bt, lhsT, m_rows, psacc, tagp,
                          pre_slab=None, post_slab=None, halves=False,
                          s_blocks=S_BLOCKS, p_blocks=P_BLOCKS,
                          collect_mms=None):
                for g in range(NG):
                    if pre_slab is not None:
                        pre_slab(g)
                    us = work.tile([128, GROUP * OWN], BF16, tag="p",
                                   name=f"u{tagp}_{g}")
                    # os bufs=4: head-0's matmuls lag the V/S pace while PE
                    # (in-order queue) interleaves Wh-block emission; extra
                    # os buffers let V run ahead into the next head instead
                    # of stalling on the slab-matmul release
                    os_ = work.tile([128, GROUP * OWN], BF16, tag="o",
                                    name=f"o{tagp}_{g}", bufs=4)
                    # halves: process the slab as two 4-block pieces so the
                    # first matmuls (and the last exp) are earlier -
                    # shortens the latency-bound head-0/output tail
                    chunks = ([(0, GROUP // 2), (GROUP // 2, GROUP)]
                              if halves else [(0, GROUP)])
                    for (q0, q1) in chunks:
                        for q in range(q0, q1):
                            jb = g * GROUP + q
                            dst = us[:, q * OWN: (q + 1) * OWN]
                            nmb = nm[:, jb * OWN: (jb + 1) * OWN]
                            r = q - q0
                            if r < s_blocks:
                                # S-routed block: z-base TT on V, +s2 fused
                                # into the Prelu bias on ScalarE
                                zt = small.tile([128, OWN], BF16, tag="zb",
                                                name=f"zb{tagp}_{g}_{q}")
                                nc.vector.tensor_tensor(zt[:], nmb, s1bt[:],
                                                        ADD)
                                nc.scalar.activation(dst, zt[:], AF.Prelu,
                                                     bias=s2col(jb),
                                                     alpha=ALPHA)
                            elif r < s_blocks + p_blocks:
                                # Pool-routed block: z-base TT on GpSimd
                                # (only TT/copy/memset are legal on Pool),
                                # +s2 fused into the Prelu bias on ScalarE
                                ztp = small.tile([128, OWN], BF16, tag="zp",
                                                 name=f"zp{tagp}_{g}_{q}")
                                nc.gpsimd.tensor_tensor(ztp[:], nmb,
                                                        s1bt[:], ADD)
                                nc.scalar.activation(dst, ztp[:], AF.Prelu,
                                                     bias=s2col(jb),
                                                     alpha=ALPHA)
                            else:
                                nc.vector._custom_dve(
                                    OP_LEAKY_Z, out=dst, in0=nmb,
                                    in1=s1bt[:], s0=s2col(jb), imm2=ALPHA)
                        nc.scalar.activation(os_[:, q0 * OWN: q1 * OWN],
                                             us[:, q0 * OWN: q1 * OWN], AF.Exp)
                        for q in range(q0, q1):
                            jb = g * GROUP + q
                            osq = os_[:, q * OWN: (q + 1) * OWN]
                            nc.tensor.matmul(
                                psacc[0: m_rows, :], lhsT(jb), osq,
                                start=(jb == 0), stop=(jb == NB - 1))
                            if collect_mms is not None:
                                # second-operand matmuls deferred: they wait
                                # on late data (the F gather); interleaving
                                # them here would stall the in-order PE
                                # queue and block the ready lhsT matmuls
                                collect_mms.append((jb, osq))
                    if post_slab is not None:
                        post_slab(g)

            def emit_who_half(tag, cs, prng=None):
                # prng: (p0, p1) partition half for a single-head (K=64)
                # contribution; None = full 128-partition c-blocks
                p0, p1 = prng if prng else (0, 128)
                who = const.tile([128, OB * 17], F32, name=f"whoown{tag}")
                for ib in range(OB):
                    pw = psM.tile([128, OWN], F32, tag="mm",
                                  name=f"pw{tag}{ib}")
                    for ci, c in enumerate(cs):
                        nc.tensor.matmul(
                            pw[:, 0:C],
                            hT_all[p0:p1, c * OWN + ib * 128: c * OWN + ib * 128 + 128],
                            wot[p0:p1, c * C: (c + 1) * C],
                            start=(ci == 0), stop=(ci == len(cs) - 1))
                    nc.vector.tensor_copy(who[:, ib * 17: ib * 17 + C],
                                          pw[:, 0:C])
                    tmp = small.tile([128, C], F32, tag="s2tmp",
                                     name=f"s2o{tag}{ib}")
                    nc.vector.scalar_tensor_tensor(
                        tmp[:], pw[:, 0:C], 1.0, a2o[:], MULT, MULT,
                        accum_out=who[:, ib * 17 + 16: ib * 17 + 17])
                return who

            def emit_gather_start(tag, src_ap, width):
                # ci out-DMA on the Pool queue, which carries ONLY DMA/CC
                # work (no arith -> no SWDGE<->arith ucode lib reloads);
                # the sync queue is blocked mid-phase by gather reads.
                ci = dram.tile([128, width], F32, name=f"cci{tag}")
                co = dram.tile([n_cores * 128, width], F32,
                               addr_space="Shared" if n_cores > 1 else "Local",
                               name=f"cco{tag}")
                nc.gpsimd.dma_start(out=ci[:], in_=src_ap)
                if n_cores > 1:
                    nc.gpsimd.collective_compute(
                        "AllGather", mybir.AluOpType.bypass,
                        replica_groups=[list(range(n_cores))],
                        ins=[ci.opt()], outs=[co.opt()])
                else:
                    nc.sync.dma_start(out=co[:], in_=ci[:])
                return co

            def emit_gather_read(tag, co, out_w, nsplit=1):
                ga = const.tile([128, out_w], F32, name=f"ga{tag}")
                src = co[:].rearrange("(g p) f -> p g f", p=128)
                step = n_cores // nsplit
                for qi in range(nsplit):
                    nc.sync.dma_start(
                        out=ga[:].rearrange("p (g f) -> p g f", g=n_cores)
                        [:, qi * step: (qi + 1) * step, :],
                        in_=src[:, qi * step: (qi + 1) * step, :])
                return ga

            # ---- phase B: hidden attention ----
            def finalize_head(h, ph):
                # 1/den = exp(-ln(den)) on ScalarE, straight from PSUM
                lnr = small.tile([65, OWN], F32, tag="lnr", name=f"lnr{h}", bufs=1)
                nc.scalar.activation(lnr[64:65, :], ph[64:65, :], AF.Ln)
                nc.scalar.activation(lnr[64:65, :], lnr[64:65, :], AF.Exp,
                                     scale=-1.0)
                rd = dram.tile([1, OWN], F32, name=f"rd{h}")
                # producer-local HWDGE triggers for ALL heads: the rd
                # trigger directly follows lnr's Exp on ScalarE (wait always
                # satisfied), while the sync queue is blocked mid-phase by
                # gather reads waiting on collectives - a sync-queued rd
                # here stalled the finalize (and V behind it) ~17us.
                dq = nc.scalar
                dq.dma_start(out=rd[:], in_=lnr[64:65, :])
                rb = small.tile([64, OWN], F32, tag="rb", name=f"rb{h}")
                dq.dma_start(out=rb[:],
                             in_=rd[0:1, :].to_broadcast([64, OWN]))
                tn = small.tile([64, OWN], F32, tag="tn", name=f"tn{h}", bufs=1)
                nc.vector.tensor_tensor(tn[:], ph[0:64, :], rb[:], MULT)
                m0 = small.tile([64, OWN], F32, tag="rb", name=f"m0{h}")
                nc.vector.tensor_scalar(m0[:], tn[:], 0.0, None,
                                        mybir.AluOpType.min)
                g_ = small.tile([64, OWN], F32, tag="g", name=f"g{h}", bufs=1)
                nc.scalar.activation(g_[:], m0[:], AF.Exp)
                slot = hT_all[(h % 2) * 64: (h % 2) * 64 + 64,
                              (h // 2) * OWN: (h // 2) * OWN + OWN]
                nc.vector.scalar_tensor_tensor(slot, g_[:], -1.0, tn[:], ADD, MAX)

            prev = [None]
            gaA_ref = [None]
            coC_ref = [None]
            coE_ref = [None]
            whoAC = const.tile([128, NB * 17], F32)
            whoACE = const.tile([128, NB * 17], F32)
            s2oX = const.tile([128, NB], F32)
            s2oY = const.tile([128, NB], F32)
            for h in range(H):
                ph = psH.tile([65, OWN], F32, tag="ph", name=f"ph{h}")

                def pre_slab(g, h=h, ph=ph):
                    if g == (1 if NG > 1 else 0) and prev[0] is not None:
                        # previous head's normalize/elu, off the critical path
                        finalize_head(h - 1, prev[0])
                    if h == 4 and g == 2:
                        # heads 0-3 finalized: gather their Wh_o contribution
                        # while heads 4-7 compute. Emitted mid-head so the
                        # pw matmuls/copies land mid-queue, not behind the
                        # whole head's work.
                        whoA = emit_who_half("A", [0, 1])
                        coA = emit_gather_start("A", whoA[:], OB * 17)
                        gaA_ref[0] = emit_gather_read("A", coA, NB * 17)
                    if h == 6 and g == 2:
                        # heads 4-5 finalized: start their gather during
                        # heads 6-7
                        whoC = emit_who_half("C", [2])
                        coC_ref[0] = emit_gather_start("C", whoC[:], OB * 17)
                    if h == 7 and g == 2:
                        # head 6 finalized (at g==1): its half of c-block 3
                        # via K=64 matmuls; gather during head 7's tail
                        whoE = emit_who_half("E", [3], prng=(0, 64))
                        coE_ref[0] = emit_gather_start("E", whoE[:], OB * 17)
                    if h == 7 and g == 3:
                        # C completed during head 6/7: read + combine with A
                        # on VectorE (GpSimd arith here would force ucode
                        # lib swaps around the collective triggers)
                        gaC = emit_gather_read("C", coC_ref[0], NB * 17)
                        gaA = gaA_ref[0]
                        nc.vector.tensor_tensor(whoAC[:], gaA[:], gaC[:], ADD)
                        nc.vector.tensor_tensor(
                            s2oX[:].rearrange("p (b w) -> p b w", w=1),
                            gaA[:].rearrange("p (b w) -> p b w", w=17)[:, :, 16:17],
                            gaC[:].rearrange("p (b w) -> p b w", w=17)[:, :, 16:17],
                            ADD)

                def post_slab(g, h=h):
                    # head 0: remaining Wh blocks two slabs ahead, after the
                    # slab's own work so the casts/matmuls don't head-of-
                    # line-block the slab's exps and attention matmuls
                    if h == 0:
                        for j in range((g + 2) * GROUP,
                                       min((g + 3) * GROUP, NB)):
                            emit_wh_block(j)

                attention(lambda jb, h=h: s2fg[jb // GROUP][:, (jb % GROUP) * 8 + h: (jb % GROUP) * 8 + h + 1],
                          s1b[h], whA(h), 65, ph, f"h{h}",
                          pre_slab=pre_slab, post_slab=post_slab,
                          halves=(h in (0, H - 1)))
                prev[0] = ph
            finalize_head(H - 1, prev[0])

            # ---- phase C tail: only head-7's half of c-block 3 (~35KB)
            # plus the tiny s2o column remain to gather. ----
            # s1ob depends only on hT_all: emit before the final collective
            s1ob = const.tile([128, OWN], BF16)
            ps1o = psM.tile([128, OWN], F32, tag="mm", name="ps1o")
            for c in range(KF):
                nc.tensor.matmul(ps1o[:], wo1[:, c * 128: (c + 1) * 128],
                                 hT_all[:, c * OWN: (c + 1) * OWN],
                                 start=(c == 0), stop=(c == KF - 1))
            nc.vector.tensor_copy(s1ob[:], ps1o[:])

            # who17 = bf16(whoACE) with the ones column (denominators);
            # who2 = bf16(gaF) with a ZERO 17th column. The A+C+E / F halves
            # are summed by the PE itself: two accumulating matmuls per
            # j-block, so no f32 assembly pass sits on the critical tail.
            who17 = const.tile([128, NB * 17], BF16)
            nc.vector.memset(
                who17[:].rearrange("p (b w) -> p b w", w=17)[:, :, 16:17], 1.0)
            who2 = const.tile([128, NB * 17], BF16)
            nc.vector.memset(
                who2[:].rearrange("p (b w) -> p b w", w=17)[:, :, 16:17], 0.0)

            whoF = emit_who_half("F", [3], prng=(64, 128))
            # collectives dispatched FIRST (tiny s2o column, then the F
            # half): nothing may queue ahead of their ci DMAs/doorbells
            coS = emit_gather_start(
                "S", whoF[:].rearrange("p (b w) -> p b w", w=17)[:, :, 16:17],
                OB)
            coF = emit_gather_start("F", whoF[:], OB * 17)
            # E-half readback + A+C+E combines on V (E completed during
            # head 7); who17 cast is ready before the F data lands
            gaE = emit_gather_read("E", coE_ref[0], NB * 17)
            nc.vector.tensor_tensor(whoACE[:], whoAC[:], gaE[:], ADD)
            nc.vector.tensor_tensor(
                s2oY[:].rearrange("p (b w) -> p b w", w=1),
                s2oX[:].rearrange("p (b w) -> p b w", w=1),
                gaE[:].rearrange("p (b w) -> p b w", w=17)[:, :, 16:17],
                ADD)
            nc.vector.tensor_copy(
                who17[:].rearrange("p (b w) -> p b w", w=17)[:, :, 0:C],
                whoACE[:].rearrange("p (b w) -> p b w", w=17)[:, :, 0:C])
            s2obF = emit_gather_read("S", coS, NB)
            s2oall = const.tile([128, NB], F32)
            nc.vector.tensor_tensor(
                s2oall[:].rearrange("p (b w) -> p b w", w=1),
                s2oY[:].rearrange("p (b w) -> p b w", w=1),
                s2obF[:].rearrange("p (b w) -> p b w", w=1),
                ADD)
            gaF = emit_gather_read("F", coF, NB * 17, nsplit=4)

            # ---- phase D: output attention, split accumulation ----
            # po1 accumulates P @ who17 (the A+C+E half, local before the F
            # collective lands) so its 32 matmuls + all the fused element-
            # wise overlap the F flight; the 32 who2 (F-half) matmuls are
            # deferred into one batch after gaF arrives. os bufs=4 keeps all
            # four output slabs' exp results alive for the deferred pass.
            po1 = psM.tile([128, OWN], F32, tag="mm", name="po1")
            po2 = psM.tile([128, OWN], F32, tag="mm", name="po2")

            who2_mms = []
            attention(lambda jb: s2oall[:, jb: jb + 1],
                      s1ob, lambda jb: who17[:, jb * 17: jb * 17 + 17],
                      17, po1, "o", halves=True, s_blocks=1, p_blocks=0,
                      collect_mms=who2_mms)
            # F-half cast on ScalarE after all the output exps (it waits on
            # the collective; queued earlier it would head-of-line-block
            # the exps on the ScalarE queue)
            nc.scalar.activation(
                who2[:].rearrange("p (b w) -> p b w", w=17)[:, :, 0:C],
                gaF[:].rearrange("p (b w) -> p b w", w=17)[:, :, 0:C],
                AF.Copy)
            for mi, (jb, osq) in enumerate(who2_mms):
                nc.tensor.matmul(po2[0:17, :],
                                 who2[:, jb * 17: jb * 17 + 17], osq,
                                 start=(mi == 0), stop=(mi == len(who2_mms) - 1))

            # ---- phase E: transpose, normalize, elu, log_softmax, store ----
            osb = const.tile([17, OWN], F32)
            nc.scalar.activation(osb[:], po1[0:17, :], AF.Copy)
            # merge the F-half accumulator (one PSUM operand per op: DVE has
            # a single PSUM read port)
            nc.vector.scalar_tensor_tensor(osb[:], po2[0:17, :], 1.0,
                                           osb[:], MULT, ADD)
            ptr = psM.tile([128, OB * 17], F32, tag="mm", name="ptr")
            for tt in range(OB):
                nc.tensor.transpose(ptr[:, tt * 17: tt * 17 + 17],
                                    osb[0:17, tt * 128: (tt + 1) * 128],
                                    ident[0:17, 0:17])
            es = const.tile([128, OB * 17], F32)
            nc.vector.tensor_copy(es[:], ptr[:])
            rec4 = const.tile([128, OB], F32)
            nc.vector.reciprocal(
                rec4[:], es[:].rearrange("p (b w) -> p b w", w=17)[:, :, 16:17])
            # batched normalize / elu / log_softmax over all OB row-blocks
            t1 = const.tile([128, OB * C], F32)
            nc.vector.tensor_tensor(
                t1[:].rearrange("p (b w) -> p b w", w=C),
                es[:].rearrange("p (b w) -> p b w", w=17)[:, :, 0:C],
                rec4[:].rearrange("p (b w) -> p b w", w=1)
                .to_broadcast([128, OB, C]),
                MULT)
            m1 = const.tile([128, OB * C], F32)
            nc.vector.tensor_scalar(m1[:], t1[:], 0.0, None,
                                    mybir.AluOpType.min)
            g1 = const.tile([128, OB * C], F32)
            nc.scalar.activation(g1[:], m1[:], AF.Exp)
            e1all = const.tile([128, OB * C], F32)
            nc.vector.scalar_tensor_tensor(e1all[:], g1[:], -1.0, t1[:],
                                           ADD, MAX)
            sall = const.tile([128, OB], F32)
            final = const.tile([128, OB * C], F32)
            for tt in range(OB):
                ex = small.tile([128, C], F32, tag="ex", name=f"ex{tt}")
                nc.scalar.activation(ex[:], e1all[:, tt * C: (tt + 1) * C],
                                     AF.Exp, accum_out=sall[:, tt: tt + 1])
            lns = const.tile([128, OB], F32)
            nc.scalar.activation(lns[:], sall[:], AF.Ln)
            nc.vector.tensor_tensor(
                final[:].rearrange("p (b w) -> p b w", w=C),
                e1all[:].rearrange("p (b w) -> p b w", w=C),
                lns[:].rearrange("p (b w) -> p b w", w=1)
                .to_broadcast([128, OB, C]),
                mybir.AluOpType.subtract)
            nc.sync.dma_start(
                out=d_out[:].rearrange("(b p) c -> p b c", p=128),
                in_=final[:])

    nc.compile()
    return nc


def _pack_inputs(x, adj, W, a, W_out, a_out, n_cores=N_CORES):
    """Host-side shard + layout packing. Returns list of per-core in_maps."""
    n, f_in = x.shape
    OWN = n // n_cores
    NB = n // 128
    xf = np.asarray(x, np.float32)
    adj = np.asarray(adj)
    Wf = np.asarray(W, np.float32)
    af = np.asarray(a, np.float32)
    Wof = np.asarray(W_out, np.float32)
    aof = np.asarray(a_out, np.float32)

    # xT[p, k*n + m] = x[m, 128k+p]
    xT = xf.T.reshape(KF, 128, n).transpose(1, 0, 2).reshape(128, KF * n)
    xT = xT.astype(BF)
    # k-major, head-minor weight layout: w64k[p, (k*H + h)*O + o]
    w64k = (Wf.reshape(H, KF, 128, O).transpose(2, 1, 0, 3)
            .reshape(128, KF * H * O).astype(BF))
    wa1 = np.einsum("hfo,ho->hf", Wf, af[:, :O])  # [H, F]
    wa2 = np.einsum("hfo,ho->hf", Wf, af[:, O:])
    # waA[p, k*16 + m]: m<8 -> wa2[m], else wa1[m-8]
    waA = np.concatenate([wa2, wa1], axis=0)  # [16, F]
    waA = waA.T.reshape(KF, 128, 16).transpose(1, 0, 2).reshape(128, KF * 16)
    waA = waA.astype(BF)
    wo1 = Wof @ aof[:C]  # [F]
    wo1r = np.broadcast_to(
        wo1.reshape(KF, 128).T[:, :, None], (128, KF, 128)
    ).reshape(128, KF * 128).astype(BF)
    wot = (Wof.reshape(KF, 128, C).transpose(1, 0, 2)
           .reshape(128, KF * C).astype(BF))
    a2o = np.broadcast_to(aof[C:], (128, C)).astype(np.float32).copy()

    in_maps = []
    for core in range(n_cores):
        rows = slice(core * OWN, (core + 1) * OWN)
        xo = (xf[rows].T.reshape(KF, 128, OWN).transpose(1, 0, 2)
              .reshape(128, KF * OWN).astype(BF))
        nmT = np.where(adj[rows].T > 0, np.float32(0), np.float32(NEG))
        nmT = (nmT.reshape(NB, 128, OWN).transpose(1, 0, 2)
               .reshape(128, NB * OWN).astype(BF))
        in_maps.append({
            "xT": xT, "xo": xo, "w64k": w64k, "waA": waA, "wo1r": wo1r,
            "nmT": nmT, "wot": wot, "a2o": a2o,
        })
    return in_maps


_NC_CACHE = {}


def _get_nc(n_cores=N_CORES, n=N):
    key = (n_cores, n)
    if key not in _NC_CACHE:
        _NC_CACHE[key] = _build_nc(n_cores, n)
    return _NC_CACHE[key]


def kernel(x, adj, W, a, W_out, a_out):
    nc = _get_nc()
    in_maps = _pack_inputs(x, adj, W, a, W_out, a_out)
    res = run_bass_kernel_spmd(nc, in_maps, list(range(N_CORES)))
    out = np.concatenate([res.results[c]["out"] for c in range(N_CORES)], axis=0)
    return out.astype(np.float32)
